# revision 52
# baseline (speedup 1.0000x reference)
"""Trainium2 Bass kernel for PhysicsInformedCtxLiquidNetwork (1024-step liquid NN).

Self-contained: hardcodes shapes/sharding. Accepts FULL inputs, returns FULL output.

Strategy (data-parallel over batch, 8 cores x 32 batch):
  - State kept TRANSPOSED: hAT [128, 4*32] f32r SBUF (h~ = h/DT; final LN is
    scale-invariant so LN(h~) == LN(h); clip provably inactive for this tau).
  - Per step: xa matmul is its own complete PSUM group (scheduler hoists it
    into the previous step's idle window); 4 K=128 chunk matmuls accumulate
    on top with skip_group_check. zs (row-sum col) gives S1 of the recurrent
    part; the input-projection row-sum is folded in from a host-precomputed
    SBUF constant (xpsneg) during the negmu op.
  - Variance in ONE DVE op: tensor_tensor_reduce computes (z*z)/512 summed
    with initial value (eps - mu^2) -> ve directly.
  - rstd via int32 bit-hack seed + 1 Newton iter on DVE.
  - f = tanh(z*rstd + bias2) in one ACT op; 4 PE transposes f -> fT PSUM;
    update in transposed space: hAT' = a*hAT + fT (one DVE op, f32r out).
  - Optional PE p-state filler matmuls keep the PE busy through the
    stats/tanh window so real matmuls run at 2.4GHz instead of 1.2GHz.
"""

import math
import numpy as np
from contextlib import ExitStack

import concourse.bass as bass
import concourse.bacc as bacc
import concourse.tile as tile
from concourse import mybir
from concourse.bass_utils import run_bass_kernel_spmd

F32 = mybir.dt.float32
F32R = mybir.dt.float32r
I32 = mybir.dt.int32
AF = mybir.ActivationFunctionType
OP = mybir.AluOpType

HIDDEN = 512
INPUT = 2
CTX = 6
NAPP = 20
DT = 0.1
B_FULL = 256
N_CORES = 8
BL = B_FULL // N_CORES  # 32 per core
EPS = 1e-5
MAGIC = 0x5F3759DF

import os
N_FILLER = int(os.environ.get("K_FILLER", "0"))   # p-state fillers after z MMs
N_FILLER2 = int(os.environ.get("K_FILLER2", "0"))  # p-state fillers after transposes
NEWTON_ITERS = int(os.environ.get("K_NEWTON", "2"))
USE_CUSTOM_NR = os.environ.get("K_CUSTOM_NR", "1") == "1"
RSQRT_MODE = os.environ.get("K_RSQRT", "h3")  # "h3" or "nr"

_BUILD_CACHE = {}
_RSQRT_NR_OP = None
_RSQRT_H3_OP = None


def _get_rsqrt_h3():
    """Register (once) a custom DVE op computing a full 3rd-order Householder
    rsqrt refinement: out = y*(1 + e*(1/2 + 3/8*e)), e = 1 - x*y*y."""
    global _RSQRT_H3_OP
    if _RSQRT_H3_OP is not None:
        return _RSQRT_H3_OP
    import concourse.dve_ops as dve_ops
    from concourse.dve_spec import Spec, Src0, Src1, C0, C1, One, lower
    from concourse.dve_uop import DveOpSpec
    for op in dve_ops.OPS:
        if op.name == "RSQRT_H3_ANT":
            _RSQRT_H3_OP = op
            return op
    e = One - Src0 * (Src1 * Src1)
    body = Src1 * (One + e * (C1 + C0 * e))
    spec = Spec(
        body=body,
        reference=lambda in0, in1, s0, s1, imm2: in1
        * (1 + (1 - in0 * in1 * in1) * (s1 + s0 * (1 - in0 * in1 * in1))),
    )
    row = max(dve_ops._SUB_OPCODE_FOR_NAME.values()) + 1
    assert row < 0x20
    shas = {}
    for ver in ("v3", "v4"):
        uops = lower(spec, ver=ver)
        shas[ver] = DveOpSpec(
            name="RSQRT_H3_ANT", opcode=row, uops=uops, rd1_en=True
        ).sha(ver)
    op = dve_ops.DveOp("RSQRT_H3_ANT", spec, subdim=False, uops_sha=shas)
    dve_ops.OPS.append(op)
    dve_ops._SUB_OPCODE_FOR_NAME[op.name] = row
    dve_ops.CUSTOM_DVE_SPECS[op.name] = spec
    _RSQRT_H3_OP = op
    return op


def _get_rsqrt_nr():
    """Register (once) a custom DVE op: out = in1*(s0 - (in0*s1)*(in1*in1)),
    i.e. one Newton-Raphson rsqrt step y*(1.5 - 0.5*x*y^2) in ONE instruction."""
    global _RSQRT_NR_OP
    if _RSQRT_NR_OP is not None:
        return _RSQRT_NR_OP
    import concourse.dve_ops as dve_ops
    from concourse.dve_spec import Spec, Src0, Src1, C0, C1, lower
    from concourse.dve_uop import DveOpSpec
    for op in dve_ops.OPS:
        if op.name == "RSQRT_NR_ANT":
            _RSQRT_NR_OP = op
            return op
    body = Src1 * (C0 - (Src0 * C1) * (Src1 * Src1))
    spec = Spec(
        body=body,
        reference=lambda in0, in1, s0, s1, imm2: in1
        * (s0 - (in0 * s1) * (in1 * in1)),
    )
    row = max(dve_ops._SUB_OPCODE_FOR_NAME.values()) + 1
    assert row < 0x20
    shas = {}
    for ver in ("v3", "v4"):
        uops = lower(spec, ver=ver)
        shas[ver] = DveOpSpec(
            name="RSQRT_NR_ANT", opcode=row, uops=uops, rd1_en=True
        ).sha(ver)
    op = dve_ops.DveOp("RSQRT_NR_ANT", spec, subdim=False, uops_sha=shas)
    dve_ops.OPS.append(op)
    dve_ops._SUB_OPCODE_FOR_NAME[op.name] = row
    dve_ops.CUSTOM_DVE_SPECS[op.name] = spec
    _RSQRT_NR_OP = op
    return op


def _emit_rstd(nc, tiny, ve, tag, iters):
    """rstd = 1/sqrt(ve) via bit-hack seed + Newton iters. Returns rstd AP."""
    s = tiny.tile([BL, 1], F32, tag=f"s_{tag}")
    t1 = tiny.tile([BL, 1], F32, tag=f"t1_{tag}")
    # seed bits: (MAGIC+1) + ~(ve_i >> 1)  == MAGIC - (ve_i >> 1)
    nc.vector.tensor_scalar(
        t1.bitcast(I32), ve.bitcast(I32), 1, -1, OP.arith_shift_right, OP.bitwise_xor
    )
    nc.vector.tensor_scalar(s.bitcast(I32), t1.bitcast(I32), MAGIC + 1, None, OP.add)
    if RSQRT_MODE == "h3":
        s2 = tiny.tile([BL, 1], F32, tag=f"s2_{tag}_h3")
        nc.vector._custom_dve(
            _get_rsqrt_h3(), out=s2, in0=ve, in1=s, s0=0.375, s1=0.5)
        return s2
    if USE_CUSTOM_NR:
        for i in range(iters):
            s2 = tiny.tile([BL, 1], F32, tag=f"s2_{tag}_{i}")
            nc.vector._custom_dve(
                _get_rsqrt_nr(), out=s2, in0=ve, in1=s, s0=1.5, s1=0.5)
            s = s2
        return s
    q = tiny.tile([BL, 1], F32, tag=f"q_{tag}")
    w = tiny.tile([BL, 1], F32, tag=f"w_{tag}")
    for _ in range(iters):
        nc.vector.tensor_mul(q, s, s)
        nc.vector.scalar_tensor_tensor(w, q, -0.5, ve, OP.mult, OP.mult)
        nc.vector.scalar_tensor_tensor(s, w, 1.5, s, OP.add, OP.mult)
    return s


def _build(n_steps, gen_flags, a_val):
    key = (n_steps, gen_flags, float(a_val))
    if key in _BUILD_CACHE:
        return _BUILD_CACHE[key]
    need_intra_aff, need_tau_vec, need_clip, need_norm_aff = gen_flags

    nc = bacc.Bacc("TRN2", target_bir_lowering=False, debug=False)

    xa_d = nc.dram_tensor("xa", [INPUT + 1, n_steps * BL], F32R, kind="ExternalInput")
    wrec_d = nc.dram_tensor("wrec", [128, 4 * 514], F32R, kind="ExternalInput")
    wx_d = nc.dram_tensor("wx", [INPUT + 1, 514], F32R, kind="ExternalInput")
    a_ident_d = nc.dram_tensor("a_ident", [128, 4 * 128], F32R, kind="ExternalInput")
    ctxa_d = nc.dram_tensor("ctxa", [CTX + 1, BL], F32, kind="ExternalInput")
    cw1_d = nc.dram_tensor("cw1", [CTX + 1, 32], F32, kind="ExternalInput")
    cw2_d = nc.dram_tensor("cw2", [33, HIDDEN], F32, kind="ExternalInput")
    hw_d = nc.dram_tensor("hw", [128, 4 * NAPP], F32R, kind="ExternalInput")
    hb_d = nc.dram_tensor("hb", [1, NAPP], F32, kind="ExternalInput")
    ident_d = nc.dram_tensor("ident", [32, 32], F32, kind="ExternalInput")
    ident128_d = nc.dram_tensor("ident128", [128, 128], F32, kind="ExternalInput")
    # general-path per-element params
    if need_intra_aff:
        ig_d = nc.dram_tensor("ig_rep", [BL, HIDDEN], F32, kind="ExternalInput")
        ib_d = nc.dram_tensor("ib_rep", [BL, HIDDEN], F32, kind="ExternalInput")
    if need_norm_aff:
        ng_d = nc.dram_tensor("ng_rep", [BL, HIDDEN], F32, kind="ExternalInput")
        nb_d = nc.dram_tensor("nb_rep", [BL, HIDDEN], F32, kind="ExternalInput")
    out_d = nc.dram_tensor("out", [BL, NAPP], F32, kind="ExternalOutput")

    with tile.TileContext(nc) as tc, ExitStack() as ctx:
        const = ctx.enter_context(tc.tile_pool(name="const", bufs=1))
        state = ctx.enter_context(tc.tile_pool(name="state", bufs=2))
        work = ctx.enter_context(tc.tile_pool(name="work", bufs=2))
        tiny = ctx.enter_context(tc.tile_pool(name="tiny", bufs=2))
        psum_z = ctx.enter_context(tc.tile_pool(name="psum_z", bufs=2, space="PSUM"))
        psum_f = ctx.enter_context(tc.tile_pool(name="psum_f", bufs=1, space="PSUM"))
        psum_g = ctx.enter_context(tc.tile_pool(name="psum_g", bufs=1, space="PSUM"))
        psum_s = ctx.enter_context(tc.tile_pool(name="psum_s", bufs=2, space="PSUM"))
        if N_FILLER or N_FILLER2:
            psum_j = ctx.enter_context(
                tc.tile_pool(name="psum_j", bufs=1, space="PSUM"))

        # ---- load constants ----
        xa = const.tile([INPUT + 1, n_steps * BL], F32R)
        wrec = const.tile([128, 4 * 514], F32R)
        wx = const.tile([INPUT + 1, 514], F32R)
        a_ident = const.tile([128, 4 * 128], F32R)
        ctxa = const.tile([CTX + 1, BL], F32)
        cw1 = const.tile([CTX + 1, 32], F32)
        cw2 = const.tile([33, HIDDEN], F32)
        hw = const.tile([128, 4 * NAPP], F32R)
        hb = const.tile([1, NAPP], F32)
        ident = const.tile([32, 32], F32)
        ident128 = const.tile([128, 128], F32)
        for sb, dr in ((xa, xa_d), (wrec, wrec_d), (wx, wx_d), (a_ident, a_ident_d),
                       (ctxa, ctxa_d), (cw1, cw1_d), (cw2, cw2_d), (hw, hw_d),
                       (hb, hb_d), (ident, ident_d), (ident128, ident128_d)):
            nc.sync.dma_start(sb[:], dr[:])
        rep = {}
        if need_intra_aff:
            rep["ig"] = const.tile([BL, HIDDEN], F32)
            rep["ib"] = const.tile([BL, HIDDEN], F32)
            nc.sync.dma_start(rep["ig"][:], ig_d[:])
            nc.sync.dma_start(rep["ib"][:], ib_d[:])
        if need_norm_aff:
            rep["ng"] = const.tile([BL, HIDDEN], F32)
            rep["nb"] = const.tile([BL, HIDDEN], F32)
            nc.sync.dma_start(rep["ng"][:], ng_d[:])
            nc.sync.dma_start(rep["nb"][:], nb_d[:])
        eps_tile = const.tile([BL, 1], F32)
        nc.vector.memset(eps_tile, EPS)
        ones_row = const.tile([1, BL], F32)
        nc.vector.memset(ones_row, 1.0)

        # ---- h0 = tanh(relu([ctx,1]@cw1) @ cw2-aug); hAT0 = transpose(h0)/DT ----
        p1 = psum_s.tile([BL, 32], F32, tag="zs")
        nc.tensor.matmul(p1, ctxa[:], cw1[:], start=True, stop=True)
        r1 = work.tile([BL, 32], F32, tag="r1")
        nc.scalar.activation(r1, p1, AF.Relu)
        r1t = work.tile([33, BL], F32, tag="r1t")
        nc.vector.transpose(r1t[0:32, :], r1[:, :])
        nc.vector.memset(r1t[32:33, :], 1.0)
        p2 = psum_z.tile([BL, HIDDEN], F32, tag="z")
        nc.tensor.matmul(p2, r1t[:], cw2[:], start=True, stop=True)
        th = work.tile([BL, HIDDEN], F32, tag="f")
        nc.scalar.activation(th, p2, AF.Tanh)
        thTa = psum_f.tile([128, 2 * BL], F32, tag="fTa")
        thTb = psum_g.tile([128, 2 * BL], F32, tag="fTb")
        for c in range(4):
            dst = thTa if c < 2 else thTb
            nc.tensor.transpose(
                dst[:, (c % 2) * BL : (c % 2 + 1) * BL],
                th[:, c * 128 : (c + 1) * 128], ident)
        hAT = state.tile([128, 4 * BL], F32R, tag="hAT")
        for half, src_ps in ((0, thTa), (1, thTb)):
            tmp = work.tile([128, 2 * BL], F32, tag="hATf0")
            nc.vector.tensor_scalar(tmp, src_ps, 1.0 / DT, None, OP.mult)
            nc.vector.tensor_copy(hAT[:, half * 2 * BL : (half + 1) * 2 * BL], tmp)

        if N_FILLER or N_FILLER2:
            junk = psum_j.tile([BL, HIDDEN], F32, tag="junk")
            nc.tensor.matmul(junk, xa[:, 0:BL], wx[:, 0:HIDDEN], start=True, stop=True)

        # ---- main loop ----
        for t in range(n_steps):
            z = psum_z.tile([BL, HIDDEN], F32, tag="z")
            zs = psum_s.tile([BL, 2], F32, tag="zs")
            xa_t = xa[:, t * BL : (t + 1) * BL]
            # zs (row-sum) matmuls FIRST so negmu/bias1 compute during the
            # z phase instead of delaying SQUARE at the end
            nc.tensor.matmul(zs, xa_t, wx[:, HIDDEN : HIDDEN + 2],
                             start=True, stop=False)
            for c in range(4):
                lhsT = hAT[:, c * BL : (c + 1) * BL]
                nc.tensor.matmul(zs, lhsT, wrec[:, c * 514 + 512 : c * 514 + 514],
                                 start=False, stop=(c == 3))
            nc.tensor.matmul(z, xa_t, wx[:, 0:HIDDEN], start=True, stop=False)
            for c in range(4):
                lhsT = hAT[:, c * BL : (c + 1) * BL]
                nc.tensor.matmul(z, lhsT, wrec[:, c * 514 : c * 514 + 512],
                                 start=False, stop=(c == 3))
            for _ in range(N_FILLER):
                nc.tensor.matmul(junk, hAT[:, 0:BL], wrec[:, 0:HIDDEN],
                                 start=False, stop=False, skip_group_check=True)
            # stats: negmu from zs (includes xa part via wx rowsum cols)
            negmu = tiny.tile([BL, 1], F32, tag="negmu")
            nc.vector.tensor_scalar(negmu, zs[:, 0:1], -1.0 / HIDDEN, None, OP.mult)
            # ve = var + eps in ONE ACT op: sum of Square(z/sqrt(512) + bias1)
            # with bias1 = (negmu + sqrt(eps))/sqrt(512); cross-term vanishes.
            bias1 = tiny.tile([BL, 1], F32, tag="bias1")
            nc.vector.tensor_scalar(
                bias1, negmu, math.sqrt(EPS), 1.0 / math.sqrt(HIDDEN), OP.add, OP.mult)
            sq = work.tile([BL, HIDDEN], F32, tag="sq")
            ve = tiny.tile([BL, 1], F32, tag="ve")
            nc.scalar.activation(sq, z, AF.Square, bias=bias1,
                                 scale=1.0 / math.sqrt(HIDDEN), accum_out=ve)
            rstd = _emit_rstd(nc, tiny, ve, "m", NEWTON_ITERS)
            bias2 = tiny.tile([BL, 1], F32, tag="bias2")
            nc.vector.tensor_mul(bias2, negmu, rstd)
            f = work.tile([BL, HIDDEN], F32, tag="f")
            if not need_intra_aff:
                nc.scalar.activation(f, z, AF.Tanh, bias=bias2, scale=rstd)
            else:
                u = work.tile([BL, HIDDEN], F32, tag="u")
                nc.scalar.activation(u, z, AF.Identity, bias=bias2, scale=rstd)
                nc.vector.tensor_mul(u, u, rep["ig"])
                nc.vector.tensor_add(u, u, rep["ib"])
                nc.scalar.activation(f, u, AF.Tanh)
            # update in transposed space, fused into two half PSUM groups:
            #   fTh = (a*I)^T @ hAT-half  (hoistable: depends only on hAT)
            #   fTh += transpose(f-half)  (2 accumulating PE transposes each)
            #   hAT'-half = copy(fTh) rounded to f32r
            # Half A's copy overlaps half B's transposes; the next step's
            # first z matmuls can start as soon as half A's copy lands.
            fTa = psum_f.tile([128, 2 * BL], F32, tag="fTa")
            fTb = psum_g.tile([128, 2 * BL], F32, tag="fTb")
            hAT_new = state.tile([128, 4 * BL], F32R, tag="hAT")
            for half, fh in ((0, fTa), (1, fTb)):
                lo = half * 2 * BL
                if not need_tau_vec:
                    nc.tensor.matmul(fh[:], a_ident[:, 0:128],
                                     hAT[:, lo : lo + 2 * BL],
                                     start=True, stop=False)
                else:
                    for cc in range(2):
                        c = half * 2 + cc
                        nc.tensor.matmul(
                            fh[:, cc * BL : (cc + 1) * BL],
                            a_ident[:, c * 128 : (c + 1) * 128],
                            hAT[:, c * BL : (c + 1) * BL], start=True, stop=False,
                            skip_group_check=(cc > 0))
                for cc in range(2):
                    c = half * 2 + cc
                    nc.tensor.matmul(
                        fh[:, cc * BL : (cc + 1) * BL],
                        f[:, c * 128 : (c + 1) * 128],
                        ident, is_transpose=True, start=False, stop=(cc == 1))
                if need_clip:
                    hnf = work.tile([128, 2 * BL], F32, tag=f"hATf{half}")
                    nc.vector.tensor_scalar(hnf, fh, 10.0 / DT, -10.0 / DT,
                                            OP.min, OP.max)
                    nc.vector.tensor_copy(hAT_new[:, lo : lo + 2 * BL], hnf)
                else:
                    nc.vector.tensor_copy(hAT_new[:, lo : lo + 2 * BL], fh[:])
            for _ in range(N_FILLER2):
                nc.tensor.matmul(junk, hAT[:, 0:BL], wrec[:, 0:HIDDEN],
                                 start=False, stop=False, skip_group_check=True)
            hAT = hAT_new

        # ---- final: transpose state back (PE, ident128), LN + head ----
        hA_ps = psum_z.tile([BL, HIDDEN], F32, tag="z")
        for c in range(4):
            nc.tensor.transpose(
                hA_ps[:, c * 128 : (c + 1) * 128],
                hAT.bitcast(F32)[:, c * BL : (c + 1) * BL], ident128)
        hA = work.tile([BL, HIDDEN], F32, tag="hA")
        nc.vector.tensor_copy(hA, hA_ps)
        S1h = tiny.tile([BL, 1], F32, tag="S1h")
        nc.vector.tensor_reduce(S1h, hA, mybir.AxisListType.X, OP.add)
        negmu = tiny.tile([BL, 1], F32, tag="negmuf")
        nc.vector.tensor_scalar(negmu, S1h, -1.0 / HIDDEN, None, OP.mult)
        sqf = work.tile([BL, HIDDEN], F32, tag="sq")
        Qf = tiny.tile([BL, 1], F32, tag="Qf")
        nc.scalar.activation(sqf, hA, AF.Square, accum_out=Qf)
        m2e = tiny.tile([BL, 1], F32, tag="m2ef")
        nc.vector.scalar_tensor_tensor(m2e, negmu, negmu, eps_tile, OP.mult,
                                       OP.subtract)
        ve = tiny.tile([BL, 1], F32, tag="vef")
        nc.vector.scalar_tensor_tensor(ve, Qf, 1.0 / HIDDEN, m2e, OP.mult,
                                       OP.subtract)
        rstd = _emit_rstd(nc, tiny, ve, "f", 2)
        bias2 = tiny.tile([BL, 1], F32, tag="bias2f")
        nc.vector.tensor_mul(bias2, negmu, rstd)
        ln = work.tile([BL, HIDDEN], F32, tag="ln")
        nc.scalar.activation(ln, hA, AF.Identity, bias=bias2, scale=rstd)
        if need_norm_aff:
            nc.vector.tensor_mul(ln, ln, rep["ng"])
            nc.vector.tensor_add(ln, ln, rep["nb"])
        lnTa_ps = psum_f.tile([128, 2 * BL], F32, tag="fTa")
        lnTb_ps = psum_g.tile([128, 2 * BL], F32, tag="fTb")
        lnT = state.tile([128, 4 * BL], F32R, tag="hAT")
        for c in range(4):
            dst = lnTa_ps if c < 2 else lnTb_ps
            nc.tensor.transpose(
                dst[:, (c % 2) * BL : (c % 2 + 1) * BL],
                ln[:, c * 128 : (c + 1) * 128], ident)
        nc.vector.tensor_copy(lnT[:, 0 : 2 * BL], lnTa_ps)
        nc.vector.tensor_copy(lnT[:, 2 * BL : 4 * BL], lnTb_ps)
        po = psum_s.tile([BL, NAPP], F32, tag="zs")
        nc.tensor.matmul(po, ones_row[:], hb[:], start=True, stop=False)
        for c in range(4):
            nc.tensor.matmul(po, lnT[:, c * BL : (c + 1) * BL],
                             hw[:, c * NAPP : (c + 1) * NAPP],
                             start=False, stop=(c == 3))
        res = work.tile([BL, NAPP], F32, tag="res")
        nc.vector.tensor_copy(res, po)
        nc.sync.dma_start(out_d[:], res[:])

    nc.compile()
    _BUILD_CACHE[key] = nc
    return nc


def _softplus(v):
    return np.log1p(np.exp(-np.abs(v))) + np.maximum(v, 0.0)


def kernel(**inputs):
    inputs = {k: np.ascontiguousarray(np.asarray(v)) for k, v in inputs.items()}
    x = inputs["x"].astype(np.float32)
    ctxv = inputs["ctx"].astype(np.float32)
    rec_w = inputs["rec_w"].astype(np.float32)
    in_w = inputs["in_w"].astype(np.float32)
    in_b = inputs["in_b"].astype(np.float32)
    tau = inputs["tau"].astype(np.float32)
    intra_g, intra_b = inputs["intra_g"].astype(np.float32), inputs["intra_b"].astype(np.float32)
    norm_g, norm_b = inputs["norm_g"].astype(np.float32), inputs["norm_b"].astype(np.float32)
    head_w, head_b = inputs["head_w"].astype(np.float32), inputs["head_b"].astype(np.float32)
    ce_w1, ce_b1 = inputs["ce_w1"].astype(np.float32), inputs["ce_b1"].astype(np.float32)
    ce_w2, ce_b2 = inputs["ce_w2"].astype(np.float32), inputs["ce_b2"].astype(np.float32)

    B, S_in, _ = x.shape
    assert B == B_FULL, B

    tau_sp = _softplus(tau).astype(np.float32)
    a_vec = (np.float32(1.0) - np.float32(DT) / tau_sp).astype(np.float32)
    need_tau_vec = not bool(np.all(a_vec == a_vec[0]))
    need_clip = not bool(np.all(tau_sp <= 10.0) and np.all(tau_sp >= DT))
    need_intra_aff = not (np.all(intra_g == 1.0) and np.all(intra_b == 0.0))
    need_norm_aff = not (np.all(norm_g == 1.0) and np.all(norm_b == 0.0))
    gen_flags = (need_intra_aff, need_tau_vec, need_clip, need_norm_aff)
    a_val = float(a_vec[0])

    nc = _build(S_in, gen_flags, a_val)

    # ---- host-side constant prep ----
    Wd = (rec_w * np.float32(DT)).astype(np.float32)  # z = h~ @ (DT*W) + x@in_w + in_b
    wrec = np.zeros((128, 4 * 514), np.float32)
    for c in range(4):
        blk = Wd[c * 128 : (c + 1) * 128, :]
        wrec[:, c * 514 : c * 514 + 512] = blk
        wrec[:, c * 514 + 512] = blk.sum(axis=1)
    wx = np.zeros((INPUT + 1, 514), np.float32)
    wx[0:INPUT, 0:HIDDEN] = in_w
    wx[INPUT, 0:HIDDEN] = in_b
    wx[0:INPUT, HIDDEN] = in_w.sum(axis=1)
    wx[INPUT, HIDDEN] = in_b.sum()
    cw1 = np.concatenate([ce_w1, ce_b1[None, :]], axis=0).astype(np.float32)  # [7,32]
    cw2 = np.concatenate([ce_w2, ce_b2[None, :]], axis=0).astype(np.float32)  # [33,512]
    hw = np.zeros((128, 4 * NAPP), np.float32)
    for c in range(4):
        hw[:, c * NAPP : (c + 1) * NAPP] = head_w[c * 128 : (c + 1) * 128, :]
    hb = head_b[None, :].astype(np.float32)
    ident = np.eye(32, dtype=np.float32)
    ident128 = np.eye(128, dtype=np.float32)
    # 4 diag blocks (diag(a_vec) per hidden chunk): fT init = (a*I)^T @ hAT
    a_ident = np.zeros((128, 4 * 128), np.float32)
    for c in range(4):
        a_ident[:, c * 128 : (c + 1) * 128] = np.diag(a_vec[c * 128 : (c + 1) * 128])

    xt = np.transpose(x, (2, 1, 0))  # [2, S, B]
    in_maps = []
    for core in range(N_CORES):
        sl = slice(core * BL, (core + 1) * BL)
        xa = np.ones((INPUT + 1, S_in * BL), np.float32)
        xa[0:INPUT] = xt[:, :, sl].reshape(INPUT, S_in * BL)
        ctxa = np.ones((CTX + 1, BL), np.float32)
        ctxa[0:CTX] = ctxv[sl].T
        m = {
            "xa": xa, "wrec": wrec, "wx": wx, "a_ident": a_ident, "ctxa": ctxa,
            "cw1": cw1, "cw2": cw2, "hw": hw, "hb": hb, "ident": ident,
            "ident128": ident128,
        }
        if need_intra_aff:
            m["ig_rep"] = np.broadcast_to(intra_g, (BL, HIDDEN)).copy()
            m["ib_rep"] = np.broadcast_to(intra_b, (BL, HIDDEN)).copy()
        if need_norm_aff:
            m["ng_rep"] = np.broadcast_to(norm_g, (BL, HIDDEN)).copy()
            m["nb_rep"] = np.broadcast_to(norm_b, (BL, HIDDEN)).copy()
        in_maps.append(m)

    br = run_bass_kernel_spmd(nc, in_maps, core_ids=list(range(N_CORES)))
    out = np.concatenate([np.asarray(r["out"]) for r in br.results], axis=0)
    global _LAST_RUN
    _LAST_RUN = (nc, in_maps)
    return out.astype(np.float32)


_LAST_RUN = None


def profile_exec_time_ns():
    """Re-run the last kernel invocation with NTFF tracing; return exec ns."""
    if _LAST_RUN is None:
        return None
    nc, in_maps = _LAST_RUN
    br = run_bass_kernel_spmd(nc, in_maps, core_ids=list(range(N_CORES)), trace=True)
    return br.exec_time_ns


# revision 54
# speedup vs baseline: 1.1550x; 1.1550x over previous
"""Trainium2 Bass kernel for PhysicsInformedCtxLiquidNetwork (1024-step liquid NN).

Self-contained: hardcodes shapes/sharding. Accepts FULL inputs, returns FULL output.

Strategy (data-parallel over batch, 8 cores x 32 batch; state h~ = h/DT kept
TRANSPOSED as hAT [128, 4*32] f32r SBUF — final LN is scale-invariant so
LN(h~) == LN(h); clip provably inactive for this tau):
  - Per step: zs (row-sum-column) matmuls run FIRST so negmu/bias1 are ready
    during the z phase; then the xa matmul + 4 K=128 chunk matmuls build
    z [32,512] in PSUM. The xa matmuls for step t+1 hoist into step t's
    idle PE window automatically.
  - ve = var+eps in ONE ACT op: accum of Square(z/sqrt(512) + bias1) with
    bias1 = (negmu + sqrt(eps))/sqrt(512) — the cross-term cancels exactly.
  - rstd via int32 bit-hack seed (2 DVE ops) + a CUSTOM DVE op (registered
    at import: RSQRT_H3_ANT) doing a full 3rd-order Householder refinement
    in ONE instruction (~1e-4 max rel err).
  - f = tanh(z*rstd + bias2) in one ACT op.
  - Update fused into the fT PSUM accumulation group: a matmul against the
    constant (a*I128) f32r initializes fT = a*hAT (hoistable, depends only
    on hAT), 4 PE transposes of f ACCUMULATE on top (start=False), and one
    DVE tensor_copy rounds fT to the new f32r state.
  - PE p-state fillers: junk 512-row matmuls (10 after the z phase, 2 after
    the transposes) keep the PE continuously busy so real matmuls run near
    2.4GHz instead of the 1.2GHz mid p-state (z matmul 476ns -> ~300ns).
Measured: 7.63ms (staged baseline) -> 5.24ms, rel_err ~5.8e-3 (tol 2e-2).
"""

import math
import numpy as np
from contextlib import ExitStack

import concourse.bass as bass
import concourse.bacc as bacc
import concourse.tile as tile
from concourse import mybir
from concourse.bass_utils import run_bass_kernel_spmd

F32 = mybir.dt.float32
F32R = mybir.dt.float32r
I32 = mybir.dt.int32
AF = mybir.ActivationFunctionType
OP = mybir.AluOpType

HIDDEN = 512
INPUT = 2
CTX = 6
NAPP = 20
DT = 0.1
B_FULL = 256
N_CORES = 8
BL = B_FULL // N_CORES  # 32 per core
EPS = 1e-5
MAGIC = 0x5F3759DF

import os
N_FILLER = int(os.environ.get("K_FILLER", "10"))   # p-state fillers after z MMs
N_FILLER2 = int(os.environ.get("K_FILLER2", "2"))  # p-state fillers after transposes
NEWTON_ITERS = int(os.environ.get("K_NEWTON", "2"))
USE_CUSTOM_NR = os.environ.get("K_CUSTOM_NR", "1") == "1"
RSQRT_MODE = os.environ.get("K_RSQRT", "h3")  # "h3" or "nr"

_BUILD_CACHE = {}
_RSQRT_NR_OP = None
_RSQRT_H3_OP = None


def _get_rsqrt_h3():
    """Register (once) a custom DVE op computing a full 3rd-order Householder
    rsqrt refinement: out = y*(1 + e*(1/2 + 3/8*e)), e = 1 - x*y*y."""
    global _RSQRT_H3_OP
    if _RSQRT_H3_OP is not None:
        return _RSQRT_H3_OP
    import concourse.dve_ops as dve_ops
    from concourse.dve_spec import Spec, Src0, Src1, C0, C1, One, lower
    from concourse.dve_uop import DveOpSpec
    for op in dve_ops.OPS:
        if op.name == "RSQRT_H3_ANT":
            _RSQRT_H3_OP = op
            return op
    e = One - Src0 * (Src1 * Src1)
    body = Src1 * (One + e * (C1 + C0 * e))
    spec = Spec(
        body=body,
        reference=lambda in0, in1, s0, s1, imm2: in1
        * (1 + (1 - in0 * in1 * in1) * (s1 + s0 * (1 - in0 * in1 * in1))),
    )
    row = max(dve_ops._SUB_OPCODE_FOR_NAME.values()) + 1
    assert row < 0x20
    shas = {}
    for ver in ("v3", "v4"):
        uops = lower(spec, ver=ver)
        shas[ver] = DveOpSpec(
            name="RSQRT_H3_ANT", opcode=row, uops=uops, rd1_en=True
        ).sha(ver)
    op = dve_ops.DveOp("RSQRT_H3_ANT", spec, subdim=False, uops_sha=shas)
    dve_ops.OPS.append(op)
    dve_ops._SUB_OPCODE_FOR_NAME[op.name] = row
    dve_ops.CUSTOM_DVE_SPECS[op.name] = spec
    _RSQRT_H3_OP = op
    return op


def _get_rsqrt_nr():
    """Register (once) a custom DVE op: out = in1*(s0 - (in0*s1)*(in1*in1)),
    i.e. one Newton-Raphson rsqrt step y*(1.5 - 0.5*x*y^2) in ONE instruction."""
    global _RSQRT_NR_OP
    if _RSQRT_NR_OP is not None:
        return _RSQRT_NR_OP
    import concourse.dve_ops as dve_ops
    from concourse.dve_spec import Spec, Src0, Src1, C0, C1, lower
    from concourse.dve_uop import DveOpSpec
    for op in dve_ops.OPS:
        if op.name == "RSQRT_NR_ANT":
            _RSQRT_NR_OP = op
            return op
    body = Src1 * (C0 - (Src0 * C1) * (Src1 * Src1))
    spec = Spec(
        body=body,
        reference=lambda in0, in1, s0, s1, imm2: in1
        * (s0 - (in0 * s1) * (in1 * in1)),
    )
    row = max(dve_ops._SUB_OPCODE_FOR_NAME.values()) + 1
    assert row < 0x20
    shas = {}
    for ver in ("v3", "v4"):
        uops = lower(spec, ver=ver)
        shas[ver] = DveOpSpec(
            name="RSQRT_NR_ANT", opcode=row, uops=uops, rd1_en=True
        ).sha(ver)
    op = dve_ops.DveOp("RSQRT_NR_ANT", spec, subdim=False, uops_sha=shas)
    dve_ops.OPS.append(op)
    dve_ops._SUB_OPCODE_FOR_NAME[op.name] = row
    dve_ops.CUSTOM_DVE_SPECS[op.name] = spec
    _RSQRT_NR_OP = op
    return op


def _emit_rstd(nc, tiny, ve, tag, iters):
    """rstd = 1/sqrt(ve) via bit-hack seed + Newton iters. Returns rstd AP."""
    s = tiny.tile([BL, 1], F32, tag=f"s_{tag}")
    t1 = tiny.tile([BL, 1], F32, tag=f"t1_{tag}")
    # seed bits: (MAGIC+1) + ~(ve_i >> 1)  == MAGIC - (ve_i >> 1)
    nc.vector.tensor_scalar(
        t1.bitcast(I32), ve.bitcast(I32), 1, -1, OP.arith_shift_right, OP.bitwise_xor
    )
    nc.vector.tensor_scalar(s.bitcast(I32), t1.bitcast(I32), MAGIC + 1, None, OP.add)
    if RSQRT_MODE == "h3":
        s2 = tiny.tile([BL, 1], F32, tag=f"s2_{tag}_h3")
        nc.vector._custom_dve(
            _get_rsqrt_h3(), out=s2, in0=ve, in1=s, s0=0.375, s1=0.5)
        return s2
    if USE_CUSTOM_NR:
        for i in range(iters):
            s2 = tiny.tile([BL, 1], F32, tag=f"s2_{tag}_{i}")
            nc.vector._custom_dve(
                _get_rsqrt_nr(), out=s2, in0=ve, in1=s, s0=1.5, s1=0.5)
            s = s2
        return s
    q = tiny.tile([BL, 1], F32, tag=f"q_{tag}")
    w = tiny.tile([BL, 1], F32, tag=f"w_{tag}")
    for _ in range(iters):
        nc.vector.tensor_mul(q, s, s)
        nc.vector.scalar_tensor_tensor(w, q, -0.5, ve, OP.mult, OP.mult)
        nc.vector.scalar_tensor_tensor(s, w, 1.5, s, OP.add, OP.mult)
    return s


def _build(n_steps, gen_flags, a_val):
    key = (n_steps, gen_flags, float(a_val))
    if key in _BUILD_CACHE:
        return _BUILD_CACHE[key]
    need_intra_aff, need_tau_vec, need_clip, need_norm_aff = gen_flags

    nc = bacc.Bacc("TRN2", target_bir_lowering=False, debug=False)

    xa_d = nc.dram_tensor("xa", [INPUT + 1, n_steps * BL], F32R, kind="ExternalInput")
    wrec_d = nc.dram_tensor("wrec", [128, 4 * 514], F32R, kind="ExternalInput")
    wx_d = nc.dram_tensor("wx", [INPUT + 1, 514], F32R, kind="ExternalInput")
    a_ident_d = nc.dram_tensor("a_ident", [128, 4 * 128], F32R, kind="ExternalInput")
    ctxa_d = nc.dram_tensor("ctxa", [CTX + 1, BL], F32, kind="ExternalInput")
    cw1_d = nc.dram_tensor("cw1", [CTX + 1, 32], F32, kind="ExternalInput")
    cw2_d = nc.dram_tensor("cw2", [33, HIDDEN], F32, kind="ExternalInput")
    hw_d = nc.dram_tensor("hw", [128, 4 * NAPP], F32R, kind="ExternalInput")
    hb_d = nc.dram_tensor("hb", [1, NAPP], F32, kind="ExternalInput")
    ident_d = nc.dram_tensor("ident", [32, 32], F32, kind="ExternalInput")
    ident128_d = nc.dram_tensor("ident128", [128, 128], F32, kind="ExternalInput")
    # general-path per-element params
    if need_intra_aff:
        ig_d = nc.dram_tensor("ig_rep", [BL, HIDDEN], F32, kind="ExternalInput")
        ib_d = nc.dram_tensor("ib_rep", [BL, HIDDEN], F32, kind="ExternalInput")
    if need_norm_aff:
        ng_d = nc.dram_tensor("ng_rep", [BL, HIDDEN], F32, kind="ExternalInput")
        nb_d = nc.dram_tensor("nb_rep", [BL, HIDDEN], F32, kind="ExternalInput")
    out_d = nc.dram_tensor("out", [BL, NAPP], F32, kind="ExternalOutput")

    with tile.TileContext(nc) as tc, ExitStack() as ctx:
        const = ctx.enter_context(tc.tile_pool(name="const", bufs=1))
        state = ctx.enter_context(tc.tile_pool(name="state", bufs=2))
        work = ctx.enter_context(tc.tile_pool(name="work", bufs=2))
        tiny = ctx.enter_context(tc.tile_pool(name="tiny", bufs=2))
        psum_z = ctx.enter_context(tc.tile_pool(name="psum_z", bufs=2, space="PSUM"))
        psum_f = ctx.enter_context(tc.tile_pool(name="psum_f", bufs=2, space="PSUM"))
        psum_s = ctx.enter_context(tc.tile_pool(name="psum_s", bufs=2, space="PSUM"))
        if N_FILLER or N_FILLER2:
            psum_j = ctx.enter_context(
                tc.tile_pool(name="psum_j", bufs=1, space="PSUM"))

        # ---- load constants ----
        xa = const.tile([INPUT + 1, n_steps * BL], F32R)
        wrec = const.tile([128, 4 * 514], F32R)
        wx = const.tile([INPUT + 1, 514], F32R)
        a_ident = const.tile([128, 4 * 128], F32R)
        ctxa = const.tile([CTX + 1, BL], F32)
        cw1 = const.tile([CTX + 1, 32], F32)
        cw2 = const.tile([33, HIDDEN], F32)
        hw = const.tile([128, 4 * NAPP], F32R)
        hb = const.tile([1, NAPP], F32)
        ident = const.tile([32, 32], F32)
        ident128 = const.tile([128, 128], F32)
        for sb, dr in ((xa, xa_d), (wrec, wrec_d), (wx, wx_d), (a_ident, a_ident_d),
                       (ctxa, ctxa_d), (cw1, cw1_d), (cw2, cw2_d), (hw, hw_d),
                       (hb, hb_d), (ident, ident_d), (ident128, ident128_d)):
            nc.sync.dma_start(sb[:], dr[:])
        rep = {}
        if need_intra_aff:
            rep["ig"] = const.tile([BL, HIDDEN], F32)
            rep["ib"] = const.tile([BL, HIDDEN], F32)
            nc.sync.dma_start(rep["ig"][:], ig_d[:])
            nc.sync.dma_start(rep["ib"][:], ib_d[:])
        if need_norm_aff:
            rep["ng"] = const.tile([BL, HIDDEN], F32)
            rep["nb"] = const.tile([BL, HIDDEN], F32)
            nc.sync.dma_start(rep["ng"][:], ng_d[:])
            nc.sync.dma_start(rep["nb"][:], nb_d[:])
        eps_tile = const.tile([BL, 1], F32)
        nc.vector.memset(eps_tile, EPS)
        ones_row = const.tile([1, BL], F32)
        nc.vector.memset(ones_row, 1.0)

        # ---- h0 = tanh(relu([ctx,1]@cw1) @ cw2-aug); hAT0 = transpose(h0)/DT ----
        p1 = psum_s.tile([BL, 32], F32, tag="zs")
        nc.tensor.matmul(p1, ctxa[:], cw1[:], start=True, stop=True)
        r1 = work.tile([BL, 32], F32, tag="r1")
        nc.scalar.activation(r1, p1, AF.Relu)
        r1t = work.tile([33, BL], F32, tag="r1t")
        nc.vector.transpose(r1t[0:32, :], r1[:, :])
        nc.vector.memset(r1t[32:33, :], 1.0)
        p2 = psum_z.tile([BL, HIDDEN], F32, tag="z")
        nc.tensor.matmul(p2, r1t[:], cw2[:], start=True, stop=True)
        th = work.tile([BL, HIDDEN], F32, tag="f")
        nc.scalar.activation(th, p2, AF.Tanh)
        thT = psum_f.tile([128, 4 * BL], F32, tag="fT")
        for c in range(4):
            nc.tensor.transpose(
                thT[:, c * BL : (c + 1) * BL], th[:, c * 128 : (c + 1) * 128], ident
            )
        hAT0f = work.tile([128, 4 * BL], F32, tag="hATf")
        nc.vector.tensor_scalar(hAT0f, thT, 1.0 / DT, None, OP.mult)
        hAT = state.tile([128, 4 * BL], F32R, tag="hAT")
        nc.vector.tensor_copy(hAT[:], hAT0f)

        if N_FILLER or N_FILLER2:
            junk = psum_j.tile([BL, HIDDEN], F32, tag="junk")
            nc.tensor.matmul(junk, xa[:, 0:BL], wx[:, 0:HIDDEN], start=True, stop=True)

        # ---- main loop ----
        for t in range(n_steps):
            z = psum_z.tile([BL, HIDDEN], F32, tag="z")
            zs = psum_s.tile([BL, 2], F32, tag="zs")
            xa_t = xa[:, t * BL : (t + 1) * BL]
            # zs (row-sum) matmuls FIRST so negmu/bias1 compute during the
            # z phase instead of delaying SQUARE at the end
            nc.tensor.matmul(zs, xa_t, wx[:, HIDDEN : HIDDEN + 2],
                             start=True, stop=False)
            for c in range(4):
                lhsT = hAT[:, c * BL : (c + 1) * BL]
                nc.tensor.matmul(zs, lhsT, wrec[:, c * 514 + 512 : c * 514 + 514],
                                 start=False, stop=(c == 3))
            nc.tensor.matmul(z, xa_t, wx[:, 0:HIDDEN], start=True, stop=False)
            for c in range(4):
                lhsT = hAT[:, c * BL : (c + 1) * BL]
                nc.tensor.matmul(z, lhsT, wrec[:, c * 514 : c * 514 + 512],
                                 start=False, stop=(c == 3))
            for _ in range(N_FILLER):
                nc.tensor.matmul(junk, hAT[:, 0:BL], wrec[:, 0:HIDDEN],
                                 start=False, stop=False, skip_group_check=True)
            # stats: negmu from zs (includes xa part via wx rowsum cols)
            negmu = tiny.tile([BL, 1], F32, tag="negmu")
            nc.vector.tensor_scalar(negmu, zs[:, 0:1], -1.0 / HIDDEN, None, OP.mult)
            # ve = var + eps in ONE ACT op: sum of Square(z/sqrt(512) + bias1)
            # with bias1 = (negmu + sqrt(eps))/sqrt(512); cross-term vanishes.
            bias1 = tiny.tile([BL, 1], F32, tag="bias1")
            nc.vector.tensor_scalar(
                bias1, negmu, math.sqrt(EPS), 1.0 / math.sqrt(HIDDEN), OP.add, OP.mult)
            sq = work.tile([BL, HIDDEN], F32, tag="sq")
            ve = tiny.tile([BL, 1], F32, tag="ve")
            nc.scalar.activation(sq, z, AF.Square, bias=bias1,
                                 scale=1.0 / math.sqrt(HIDDEN), accum_out=ve)
            rstd = _emit_rstd(nc, tiny, ve, "m", NEWTON_ITERS)
            bias2 = tiny.tile([BL, 1], F32, tag="bias2")
            nc.vector.tensor_mul(bias2, negmu, rstd)
            f = work.tile([BL, HIDDEN], F32, tag="f")
            if not need_intra_aff:
                nc.scalar.activation(f, z, AF.Tanh, bias=bias2, scale=rstd)
            else:
                u = work.tile([BL, HIDDEN], F32, tag="u")
                nc.scalar.activation(u, z, AF.Identity, bias=bias2, scale=rstd)
                nc.vector.tensor_mul(u, u, rep["ig"])
                nc.vector.tensor_add(u, u, rep["ib"])
                nc.scalar.activation(f, u, AF.Tanh)
            # update in transposed space, fused into the fT PSUM group:
            #   fT = (a*I)^T @ hAT   (hoistable: depends only on hAT)
            #   fT += transpose(f)   (4 accumulating PE transposes)
            #   hAT' = copy(fT) rounded to f32r
            fT = psum_f.tile([128, 4 * BL], F32, tag="fT")
            if not need_tau_vec:
                nc.tensor.matmul(fT[:], a_ident[:, 0:128], hAT[:],
                                 start=True, stop=False)
            else:
                for c in range(4):
                    nc.tensor.matmul(
                        fT[:, c * BL : (c + 1) * BL],
                        a_ident[:, c * 128 : (c + 1) * 128],
                        hAT[:, c * BL : (c + 1) * BL], start=True, stop=False,
                        skip_group_check=(c > 0))
            for c in range(4):
                nc.tensor.matmul(
                    fT[:, c * BL : (c + 1) * BL], f[:, c * 128 : (c + 1) * 128],
                    ident, is_transpose=True, start=False, stop=(c == 3))
            hAT_new = state.tile([128, 4 * BL], F32R, tag="hAT")
            if need_clip:
                hAT_newf = work.tile([128, 4 * BL], F32, tag="hATf")
                nc.vector.tensor_scalar(hAT_newf, fT, 10.0 / DT, -10.0 / DT,
                                        OP.min, OP.max)
                nc.vector.tensor_copy(hAT_new[:], hAT_newf)
            else:
                nc.vector.tensor_copy(hAT_new[:], fT)
            for _ in range(N_FILLER2):
                nc.tensor.matmul(junk, hAT[:, 0:BL], wrec[:, 0:HIDDEN],
                                 start=False, stop=False, skip_group_check=True)
            hAT = hAT_new

        # ---- final: transpose state back (PE, ident128), LN + head ----
        hA_ps = psum_z.tile([BL, HIDDEN], F32, tag="z")
        for c in range(4):
            nc.tensor.transpose(
                hA_ps[:, c * 128 : (c + 1) * 128],
                hAT.bitcast(F32)[:, c * BL : (c + 1) * BL], ident128)
        hA = work.tile([BL, HIDDEN], F32, tag="hA")
        nc.vector.tensor_copy(hA, hA_ps)
        S1h = tiny.tile([BL, 1], F32, tag="S1h")
        nc.vector.tensor_reduce(S1h, hA, mybir.AxisListType.X, OP.add)
        negmu = tiny.tile([BL, 1], F32, tag="negmuf")
        nc.vector.tensor_scalar(negmu, S1h, -1.0 / HIDDEN, None, OP.mult)
        sqf = work.tile([BL, HIDDEN], F32, tag="sq")
        Qf = tiny.tile([BL, 1], F32, tag="Qf")
        nc.scalar.activation(sqf, hA, AF.Square, accum_out=Qf)
        m2e = tiny.tile([BL, 1], F32, tag="m2ef")
        nc.vector.scalar_tensor_tensor(m2e, negmu, negmu, eps_tile, OP.mult,
                                       OP.subtract)
        ve = tiny.tile([BL, 1], F32, tag="vef")
        nc.vector.scalar_tensor_tensor(ve, Qf, 1.0 / HIDDEN, m2e, OP.mult,
                                       OP.subtract)
        rstd = _emit_rstd(nc, tiny, ve, "f", 2)
        bias2 = tiny.tile([BL, 1], F32, tag="bias2f")
        nc.vector.tensor_mul(bias2, negmu, rstd)
        ln = work.tile([BL, HIDDEN], F32, tag="ln")
        nc.scalar.activation(ln, hA, AF.Identity, bias=bias2, scale=rstd)
        if need_norm_aff:
            nc.vector.tensor_mul(ln, ln, rep["ng"])
            nc.vector.tensor_add(ln, ln, rep["nb"])
        lnT_ps = psum_f.tile([128, 4 * BL], F32, tag="fT")
        lnT = state.tile([128, 4 * BL], F32R, tag="hAT")
        for c in range(4):
            nc.tensor.transpose(
                lnT_ps[:, c * BL : (c + 1) * BL], ln[:, c * 128 : (c + 1) * 128],
                ident)
        nc.vector.tensor_copy(lnT[:], lnT_ps)
        po = psum_s.tile([BL, NAPP], F32, tag="zs")
        nc.tensor.matmul(po, ones_row[:], hb[:], start=True, stop=False)
        for c in range(4):
            nc.tensor.matmul(po, lnT[:, c * BL : (c + 1) * BL],
                             hw[:, c * NAPP : (c + 1) * NAPP],
                             start=False, stop=(c == 3))
        res = work.tile([BL, NAPP], F32, tag="res")
        nc.vector.tensor_copy(res, po)
        nc.sync.dma_start(out_d[:], res[:])

    nc.compile()
    _BUILD_CACHE[key] = nc
    return nc


def _softplus(v):
    return np.log1p(np.exp(-np.abs(v))) + np.maximum(v, 0.0)


def kernel(**inputs):
    inputs = {k: np.ascontiguousarray(np.asarray(v)) for k, v in inputs.items()}
    x = inputs["x"].astype(np.float32)
    ctxv = inputs["ctx"].astype(np.float32)
    rec_w = inputs["rec_w"].astype(np.float32)
    in_w = inputs["in_w"].astype(np.float32)
    in_b = inputs["in_b"].astype(np.float32)
    tau = inputs["tau"].astype(np.float32)
    intra_g, intra_b = inputs["intra_g"].astype(np.float32), inputs["intra_b"].astype(np.float32)
    norm_g, norm_b = inputs["norm_g"].astype(np.float32), inputs["norm_b"].astype(np.float32)
    head_w, head_b = inputs["head_w"].astype(np.float32), inputs["head_b"].astype(np.float32)
    ce_w1, ce_b1 = inputs["ce_w1"].astype(np.float32), inputs["ce_b1"].astype(np.float32)
    ce_w2, ce_b2 = inputs["ce_w2"].astype(np.float32), inputs["ce_b2"].astype(np.float32)

    B, S_in, _ = x.shape
    assert B == B_FULL, B

    tau_sp = _softplus(tau).astype(np.float32)
    a_vec = (np.float32(1.0) - np.float32(DT) / tau_sp).astype(np.float32)
    need_tau_vec = not bool(np.all(a_vec == a_vec[0]))
    need_clip = not bool(np.all(tau_sp <= 10.0) and np.all(tau_sp >= DT))
    need_intra_aff = not (np.all(intra_g == 1.0) and np.all(intra_b == 0.0))
    need_norm_aff = not (np.all(norm_g == 1.0) and np.all(norm_b == 0.0))
    gen_flags = (need_intra_aff, need_tau_vec, need_clip, need_norm_aff)
    a_val = float(a_vec[0])

    nc = _build(S_in, gen_flags, a_val)

    # ---- host-side constant prep ----
    Wd = (rec_w * np.float32(DT)).astype(np.float32)  # z = h~ @ (DT*W) + x@in_w + in_b
    wrec = np.zeros((128, 4 * 514), np.float32)
    for c in range(4):
        blk = Wd[c * 128 : (c + 1) * 128, :]
        wrec[:, c * 514 : c * 514 + 512] = blk
        wrec[:, c * 514 + 512] = blk.sum(axis=1)
    wx = np.zeros((INPUT + 1, 514), np.float32)
    wx[0:INPUT, 0:HIDDEN] = in_w
    wx[INPUT, 0:HIDDEN] = in_b
    wx[0:INPUT, HIDDEN] = in_w.sum(axis=1)
    wx[INPUT, HIDDEN] = in_b.sum()
    cw1 = np.concatenate([ce_w1, ce_b1[None, :]], axis=0).astype(np.float32)  # [7,32]
    cw2 = np.concatenate([ce_w2, ce_b2[None, :]], axis=0).astype(np.float32)  # [33,512]
    hw = np.zeros((128, 4 * NAPP), np.float32)
    for c in range(4):
        hw[:, c * NAPP : (c + 1) * NAPP] = head_w[c * 128 : (c + 1) * 128, :]
    hb = head_b[None, :].astype(np.float32)
    ident = np.eye(32, dtype=np.float32)
    ident128 = np.eye(128, dtype=np.float32)
    # 4 diag blocks (diag(a_vec) per hidden chunk): fT init = (a*I)^T @ hAT
    a_ident = np.zeros((128, 4 * 128), np.float32)
    for c in range(4):
        a_ident[:, c * 128 : (c + 1) * 128] = np.diag(a_vec[c * 128 : (c + 1) * 128])

    xt = np.transpose(x, (2, 1, 0))  # [2, S, B]
    in_maps = []
    for core in range(N_CORES):
        sl = slice(core * BL, (core + 1) * BL)
        xa = np.ones((INPUT + 1, S_in * BL), np.float32)
        xa[0:INPUT] = xt[:, :, sl].reshape(INPUT, S_in * BL)
        ctxa = np.ones((CTX + 1, BL), np.float32)
        ctxa[0:CTX] = ctxv[sl].T
        m = {
            "xa": xa, "wrec": wrec, "wx": wx, "a_ident": a_ident, "ctxa": ctxa,
            "cw1": cw1, "cw2": cw2, "hw": hw, "hb": hb, "ident": ident,
            "ident128": ident128,
        }
        if need_intra_aff:
            m["ig_rep"] = np.broadcast_to(intra_g, (BL, HIDDEN)).copy()
            m["ib_rep"] = np.broadcast_to(intra_b, (BL, HIDDEN)).copy()
        if need_norm_aff:
            m["ng_rep"] = np.broadcast_to(norm_g, (BL, HIDDEN)).copy()
            m["nb_rep"] = np.broadcast_to(norm_b, (BL, HIDDEN)).copy()
        in_maps.append(m)

    br = run_bass_kernel_spmd(nc, in_maps, core_ids=list(range(N_CORES)))
    out = np.concatenate([np.asarray(r["out"]) for r in br.results], axis=0)
    global _LAST_RUN
    _LAST_RUN = (nc, in_maps)
    return out.astype(np.float32)


_LAST_RUN = None


def profile_exec_time_ns():
    """Re-run the last kernel invocation with NTFF tracing; return exec ns."""
    if _LAST_RUN is None:
        return None
    nc, in_maps = _LAST_RUN
    br = run_bass_kernel_spmd(nc, in_maps, core_ids=list(range(N_CORES)), trace=True)
    return br.exec_time_ns


# revision 55
# speedup vs baseline: 1.2224x; 1.0583x over previous
"""Trainium2 Bass kernel for PhysicsInformedCtxLiquidNetwork (1024-step liquid NN).

Self-contained: hardcodes shapes/sharding. Accepts FULL inputs, returns FULL output.

Strategy (data-parallel over batch, 8 cores x 32 batch; state h~ = h/DT kept
TRANSPOSED as hAT [128, 4*32] f32r SBUF — final LN is scale-invariant so
LN(h~) == LN(h); clip provably inactive for this tau):
  - Per step: zs (row-sum-column) matmuls run FIRST so negmu/bias1 are ready
    during the z phase; then the xa matmul + 4 K=128 chunk matmuls build
    z [32,512] in PSUM. The xa matmuls for step t+1 hoist into step t's
    idle PE window automatically.
  - ve = var+eps in ONE ACT op: accum of Square(z/sqrt(512) + bias1) with
    bias1 = (negmu + sqrt(eps))/sqrt(512) — the cross-term cancels exactly.
  - rstd via int32 bit-hack seed (2 DVE ops) + a CUSTOM DVE op (registered
    at import: RSQRT_H3_ANT) doing a full 3rd-order Householder refinement
    in ONE instruction (~1e-4 max rel err).
  - f = tanh(z*rstd + bias2) in one ACT op.
  - Update fused into the fT PSUM accumulation group: a matmul against the
    constant (a*I128) f32r initializes fT = a*hAT (hoistable, depends only
    on hAT), 4 PE transposes of f ACCUMULATE on top (start=False), and one
    DVE tensor_copy rounds fT to the new f32r state.
  - PE p-state fillers: junk 512-row matmuls (10 after the z phase, 2 after
    the transposes) keep the PE continuously busy so real matmuls run near
    2.4GHz instead of the 1.2GHz mid p-state (z matmul 476ns -> ~300ns).
Measured: 7.63ms (staged baseline) -> 5.24ms, rel_err ~5.8e-3 (tol 2e-2).
"""

import math
import numpy as np
from contextlib import ExitStack

import concourse.bass as bass
import concourse.bacc as bacc
import concourse.tile as tile
from concourse import mybir
from concourse.bass_utils import run_bass_kernel_spmd

F32 = mybir.dt.float32
F32R = mybir.dt.float32r
I32 = mybir.dt.int32
AF = mybir.ActivationFunctionType
OP = mybir.AluOpType

HIDDEN = 512
INPUT = 2
CTX = 6
NAPP = 20
DT = 0.1
B_FULL = 256
N_CORES = 8
BL = B_FULL // N_CORES  # 32 per core
EPS = 1e-5
MAGIC = 0x5F3759DF

import os
N_FILLER = int(os.environ.get("K_FILLER", "8"))   # p-state fillers after z MMs
N_FILLER2 = int(os.environ.get("K_FILLER2", "2"))  # p-state fillers after transposes
NEWTON_ITERS = int(os.environ.get("K_NEWTON", "2"))
USE_CUSTOM_NR = os.environ.get("K_CUSTOM_NR", "1") == "1"
RSQRT_MODE = os.environ.get("K_RSQRT", "h3")  # "h3" or "nr"

_BUILD_CACHE = {}
_RSQRT_NR_OP = None
_RSQRT_H3_OP = None


def _get_rsqrt_h3():
    """Register (once) a custom DVE op computing a full 3rd-order Householder
    rsqrt refinement: out = y*(1 + e*(1/2 + 3/8*e)), e = 1 - x*y*y."""
    global _RSQRT_H3_OP
    if _RSQRT_H3_OP is not None:
        return _RSQRT_H3_OP
    import concourse.dve_ops as dve_ops
    from concourse.dve_spec import Spec, Src0, Src1, C0, C1, One, lower
    from concourse.dve_uop import DveOpSpec
    for op in dve_ops.OPS:
        if op.name == "RSQRT_H3_ANT":
            _RSQRT_H3_OP = op
            return op
    e = One - Src0 * (Src1 * Src1)
    body = Src1 * (One + e * (C1 + C0 * e))
    spec = Spec(
        body=body,
        reference=lambda in0, in1, s0, s1, imm2: in1
        * (1 + (1 - in0 * in1 * in1) * (s1 + s0 * (1 - in0 * in1 * in1))),
    )
    row = max(dve_ops._SUB_OPCODE_FOR_NAME.values()) + 1
    assert row < 0x20
    shas = {}
    for ver in ("v3", "v4"):
        uops = lower(spec, ver=ver)
        shas[ver] = DveOpSpec(
            name="RSQRT_H3_ANT", opcode=row, uops=uops, rd1_en=True
        ).sha(ver)
    op = dve_ops.DveOp("RSQRT_H3_ANT", spec, subdim=False, uops_sha=shas)
    dve_ops.OPS.append(op)
    dve_ops._SUB_OPCODE_FOR_NAME[op.name] = row
    dve_ops.CUSTOM_DVE_SPECS[op.name] = spec
    _RSQRT_H3_OP = op
    return op


def _get_rsqrt_nr():
    """Register (once) a custom DVE op: out = in1*(s0 - (in0*s1)*(in1*in1)),
    i.e. one Newton-Raphson rsqrt step y*(1.5 - 0.5*x*y^2) in ONE instruction."""
    global _RSQRT_NR_OP
    if _RSQRT_NR_OP is not None:
        return _RSQRT_NR_OP
    import concourse.dve_ops as dve_ops
    from concourse.dve_spec import Spec, Src0, Src1, C0, C1, lower
    from concourse.dve_uop import DveOpSpec
    for op in dve_ops.OPS:
        if op.name == "RSQRT_NR_ANT":
            _RSQRT_NR_OP = op
            return op
    body = Src1 * (C0 - (Src0 * C1) * (Src1 * Src1))
    spec = Spec(
        body=body,
        reference=lambda in0, in1, s0, s1, imm2: in1
        * (s0 - (in0 * s1) * (in1 * in1)),
    )
    row = max(dve_ops._SUB_OPCODE_FOR_NAME.values()) + 1
    assert row < 0x20
    shas = {}
    for ver in ("v3", "v4"):
        uops = lower(spec, ver=ver)
        shas[ver] = DveOpSpec(
            name="RSQRT_NR_ANT", opcode=row, uops=uops, rd1_en=True
        ).sha(ver)
    op = dve_ops.DveOp("RSQRT_NR_ANT", spec, subdim=False, uops_sha=shas)
    dve_ops.OPS.append(op)
    dve_ops._SUB_OPCODE_FOR_NAME[op.name] = row
    dve_ops.CUSTOM_DVE_SPECS[op.name] = spec
    _RSQRT_NR_OP = op
    return op


def _emit_rstd(nc, tiny, ve, tag, iters):
    """rstd = 1/sqrt(ve) via bit-hack seed + Newton iters. Returns rstd AP."""
    s = tiny.tile([BL, 1], F32, tag=f"s_{tag}")
    t1 = tiny.tile([BL, 1], F32, tag=f"t1_{tag}")
    # seed bits: (MAGIC+1) + ~(ve_i >> 1)  == MAGIC - (ve_i >> 1)
    nc.vector.tensor_scalar(
        t1.bitcast(I32), ve.bitcast(I32), 1, -1, OP.arith_shift_right, OP.bitwise_xor
    )
    nc.vector.tensor_scalar(s.bitcast(I32), t1.bitcast(I32), MAGIC + 1, None, OP.add)
    if RSQRT_MODE == "h3":
        s2 = tiny.tile([BL, 1], F32, tag=f"s2_{tag}_h3")
        nc.vector._custom_dve(
            _get_rsqrt_h3(), out=s2, in0=ve, in1=s, s0=0.375, s1=0.5)
        return s2
    if USE_CUSTOM_NR:
        for i in range(iters):
            s2 = tiny.tile([BL, 1], F32, tag=f"s2_{tag}_{i}")
            nc.vector._custom_dve(
                _get_rsqrt_nr(), out=s2, in0=ve, in1=s, s0=1.5, s1=0.5)
            s = s2
        return s
    q = tiny.tile([BL, 1], F32, tag=f"q_{tag}")
    w = tiny.tile([BL, 1], F32, tag=f"w_{tag}")
    for _ in range(iters):
        nc.vector.tensor_mul(q, s, s)
        nc.vector.scalar_tensor_tensor(w, q, -0.5, ve, OP.mult, OP.mult)
        nc.vector.scalar_tensor_tensor(s, w, 1.5, s, OP.add, OP.mult)
    return s


def _build(n_steps, gen_flags, a_val):
    key = (n_steps, gen_flags, float(a_val))
    if key in _BUILD_CACHE:
        return _BUILD_CACHE[key]
    need_intra_aff, need_tau_vec, need_clip, need_norm_aff = gen_flags

    nc = bacc.Bacc("TRN2", target_bir_lowering=False, debug=False)

    xa_d = nc.dram_tensor("xa", [INPUT + 1, n_steps * BL], F32R, kind="ExternalInput")
    wrec_d = nc.dram_tensor("wrec", [128, 4 * 514], F32R, kind="ExternalInput")
    wx_d = nc.dram_tensor("wx", [INPUT + 1, 514], F32R, kind="ExternalInput")
    a_ident_d = nc.dram_tensor("a_ident", [128, 4 * 128], F32R, kind="ExternalInput")
    ctxa_d = nc.dram_tensor("ctxa", [CTX + 1, BL], F32, kind="ExternalInput")
    cw1_d = nc.dram_tensor("cw1", [CTX + 1, 32], F32, kind="ExternalInput")
    cw2_d = nc.dram_tensor("cw2", [33, HIDDEN], F32, kind="ExternalInput")
    hw_d = nc.dram_tensor("hw", [128, 4 * NAPP], F32R, kind="ExternalInput")
    hb_d = nc.dram_tensor("hb", [1, NAPP], F32, kind="ExternalInput")
    ident_d = nc.dram_tensor("ident", [32, 32], F32, kind="ExternalInput")
    ident128_d = nc.dram_tensor("ident128", [128, 128], F32, kind="ExternalInput")
    # general-path per-element params
    if need_intra_aff:
        ig_d = nc.dram_tensor("ig_rep", [BL, HIDDEN], F32, kind="ExternalInput")
        ib_d = nc.dram_tensor("ib_rep", [BL, HIDDEN], F32, kind="ExternalInput")
    if need_norm_aff:
        ng_d = nc.dram_tensor("ng_rep", [BL, HIDDEN], F32, kind="ExternalInput")
        nb_d = nc.dram_tensor("nb_rep", [BL, HIDDEN], F32, kind="ExternalInput")
    out_d = nc.dram_tensor("out", [BL, NAPP], F32, kind="ExternalOutput")

    with tile.TileContext(nc) as tc, ExitStack() as ctx:
        const = ctx.enter_context(tc.tile_pool(name="const", bufs=1))
        state = ctx.enter_context(tc.tile_pool(name="state", bufs=2))
        work = ctx.enter_context(tc.tile_pool(name="work", bufs=2))
        tiny = ctx.enter_context(tc.tile_pool(name="tiny", bufs=2))
        psum_z = ctx.enter_context(tc.tile_pool(name="psum_z", bufs=2, space="PSUM"))
        psum_f = ctx.enter_context(tc.tile_pool(name="psum_f", bufs=2, space="PSUM"))
        psum_s = ctx.enter_context(tc.tile_pool(name="psum_s", bufs=2, space="PSUM"))
        if N_FILLER or N_FILLER2:
            psum_j = ctx.enter_context(
                tc.tile_pool(name="psum_j", bufs=1, space="PSUM"))

        # ---- load constants ----
        xa = const.tile([INPUT + 1, n_steps * BL], F32R)
        wrec = const.tile([128, 4 * 514], F32R)
        wx = const.tile([INPUT + 1, 514], F32R)
        a_ident = const.tile([128, 4 * 128], F32R)
        ctxa = const.tile([CTX + 1, BL], F32)
        cw1 = const.tile([CTX + 1, 32], F32)
        cw2 = const.tile([33, HIDDEN], F32)
        hw = const.tile([128, 4 * NAPP], F32R)
        hb = const.tile([1, NAPP], F32)
        ident = const.tile([32, 32], F32)
        ident128 = const.tile([128, 128], F32)
        for sb, dr in ((xa, xa_d), (wrec, wrec_d), (wx, wx_d), (a_ident, a_ident_d),
                       (ctxa, ctxa_d), (cw1, cw1_d), (cw2, cw2_d), (hw, hw_d),
                       (hb, hb_d), (ident, ident_d), (ident128, ident128_d)):
            nc.sync.dma_start(sb[:], dr[:])
        rep = {}
        if need_intra_aff:
            rep["ig"] = const.tile([BL, HIDDEN], F32)
            rep["ib"] = const.tile([BL, HIDDEN], F32)
            nc.sync.dma_start(rep["ig"][:], ig_d[:])
            nc.sync.dma_start(rep["ib"][:], ib_d[:])
        if need_norm_aff:
            rep["ng"] = const.tile([BL, HIDDEN], F32)
            rep["nb"] = const.tile([BL, HIDDEN], F32)
            nc.sync.dma_start(rep["ng"][:], ng_d[:])
            nc.sync.dma_start(rep["nb"][:], nb_d[:])
        eps_tile = const.tile([BL, 1], F32)
        nc.vector.memset(eps_tile, EPS)
        ones_row = const.tile([1, BL], F32)
        nc.vector.memset(ones_row, 1.0)

        # ---- h0 = tanh(relu([ctx,1]@cw1) @ cw2-aug); hAT0 = transpose(h0)/DT ----
        p1 = psum_s.tile([BL, 32], F32, tag="zs")
        nc.tensor.matmul(p1, ctxa[:], cw1[:], start=True, stop=True)
        r1 = work.tile([BL, 32], F32, tag="r1")
        nc.scalar.activation(r1, p1, AF.Relu)
        r1t = work.tile([33, BL], F32, tag="r1t")
        nc.vector.transpose(r1t[0:32, :], r1[:, :])
        nc.vector.memset(r1t[32:33, :], 1.0)
        p2 = psum_z.tile([BL, HIDDEN], F32, tag="z")
        nc.tensor.matmul(p2, r1t[:], cw2[:], start=True, stop=True)
        th = work.tile([BL, HIDDEN], F32, tag="f")
        nc.scalar.activation(th, p2, AF.Tanh)
        thT = psum_f.tile([128, 4 * BL], F32, tag="fT")
        for c in range(4):
            nc.tensor.transpose(
                thT[:, c * BL : (c + 1) * BL], th[:, c * 128 : (c + 1) * 128], ident
            )
        hAT0f = work.tile([128, 4 * BL], F32, tag="hATf")
        nc.vector.tensor_scalar(hAT0f, thT, 1.0 / DT, None, OP.mult)
        hAT = state.tile([128, 4 * BL], F32R, tag="hAT")
        nc.vector.tensor_copy(hAT[:], hAT0f)

        if N_FILLER or N_FILLER2:
            junk = psum_j.tile([BL, HIDDEN], F32, tag="junk")
            nc.tensor.matmul(junk, xa[:, 0:BL], wx[:, 0:HIDDEN], start=True, stop=True)

        # ---- main loop ----
        for t in range(n_steps):
            z = psum_z.tile([BL, HIDDEN], F32, tag="z")
            zs = psum_s.tile([BL, 2], F32, tag="zs")
            xa_t = xa[:, t * BL : (t + 1) * BL]
            # zs (row-sum) matmuls FIRST so negmu/bias1 compute during the
            # z phase instead of delaying SQUARE at the end
            nc.tensor.matmul(zs, xa_t, wx[:, HIDDEN : HIDDEN + 2],
                             start=True, stop=False)
            for c in range(4):
                lhsT = hAT[:, c * BL : (c + 1) * BL]
                nc.tensor.matmul(zs, lhsT, wrec[:, c * 514 + 512 : c * 514 + 514],
                                 start=False, stop=(c == 3))
            nc.tensor.matmul(z, xa_t, wx[:, 0:HIDDEN], start=True, stop=False)
            for c in range(4):
                lhsT = hAT[:, c * BL : (c + 1) * BL]
                nc.tensor.matmul(z, lhsT, wrec[:, c * 514 : c * 514 + 512],
                                 start=False, stop=(c == 3))
            for _ in range(N_FILLER):
                nc.tensor.matmul(junk, hAT[:, 0:BL], wrec[:, 0:HIDDEN],
                                 start=False, stop=False, skip_group_check=True)
            # stats: negmu from zs (includes xa part via wx rowsum cols)
            negmu = tiny.tile([BL, 1], F32, tag="negmu")
            nc.vector.tensor_scalar(negmu, zs[:, 0:1], -1.0 / HIDDEN, None, OP.mult)
            # ve = var + eps in ONE ACT op: sum of Square(z/sqrt(512) + bias1)
            # with bias1 = (negmu + sqrt(eps))/sqrt(512); cross-term vanishes.
            bias1 = tiny.tile([BL, 1], F32, tag="bias1")
            nc.vector.tensor_scalar(
                bias1, negmu, math.sqrt(EPS), 1.0 / math.sqrt(HIDDEN), OP.add, OP.mult)
            sq = work.tile([BL, HIDDEN], F32, tag="sq")
            ve = tiny.tile([BL, 1], F32, tag="ve")
            nc.scalar.activation(sq, z, AF.Square, bias=bias1,
                                 scale=1.0 / math.sqrt(HIDDEN), accum_out=ve)
            rstd = _emit_rstd(nc, tiny, ve, "m", NEWTON_ITERS)
            bias2 = tiny.tile([BL, 1], F32, tag="bias2")
            nc.vector.tensor_mul(bias2, negmu, rstd)
            f = work.tile([BL, HIDDEN], F32, tag="f")
            if not need_intra_aff:
                nc.scalar.activation(f, z, AF.Tanh, bias=bias2, scale=rstd)
            else:
                u = work.tile([BL, HIDDEN], F32, tag="u")
                nc.scalar.activation(u, z, AF.Identity, bias=bias2, scale=rstd)
                nc.vector.tensor_mul(u, u, rep["ig"])
                nc.vector.tensor_add(u, u, rep["ib"])
                nc.scalar.activation(f, u, AF.Tanh)
            # update in transposed space, fused into the fT PSUM group:
            #   fT = (a*I)^T @ hAT   (hoistable: depends only on hAT)
            #   fT += transpose(f)   (4 accumulating PE transposes)
            #   hAT' = copy(fT) rounded to f32r
            fT = psum_f.tile([128, 4 * BL], F32, tag="fT")
            if not need_tau_vec:
                nc.tensor.matmul(fT[:], a_ident[:, 0:128], hAT[:],
                                 start=True, stop=False)
            else:
                for c in range(4):
                    nc.tensor.matmul(
                        fT[:, c * BL : (c + 1) * BL],
                        a_ident[:, c * 128 : (c + 1) * 128],
                        hAT[:, c * BL : (c + 1) * BL], start=True, stop=False,
                        skip_group_check=(c > 0))
            for c in range(4):
                nc.tensor.matmul(
                    fT[:, c * BL : (c + 1) * BL], f[:, c * 128 : (c + 1) * 128],
                    ident, is_transpose=True, start=False, stop=(c == 3))
            hAT_new = state.tile([128, 4 * BL], F32R, tag="hAT")
            if need_clip:
                hAT_newf = work.tile([128, 4 * BL], F32, tag="hATf")
                nc.vector.tensor_scalar(hAT_newf, fT, 10.0 / DT, -10.0 / DT,
                                        OP.min, OP.max)
                nc.vector.tensor_copy(hAT_new[:], hAT_newf)
            else:
                nc.vector.tensor_copy(hAT_new[:], fT)
            for _ in range(N_FILLER2):
                nc.tensor.matmul(junk, hAT[:, 0:BL], wrec[:, 0:HIDDEN],
                                 start=False, stop=False, skip_group_check=True)
            hAT = hAT_new

        # ---- final: transpose state back (PE, ident128), LN + head ----
        hA_ps = psum_z.tile([BL, HIDDEN], F32, tag="z")
        for c in range(4):
            nc.tensor.transpose(
                hA_ps[:, c * 128 : (c + 1) * 128],
                hAT.bitcast(F32)[:, c * BL : (c + 1) * BL], ident128)
        hA = work.tile([BL, HIDDEN], F32, tag="hA")
        nc.vector.tensor_copy(hA, hA_ps)
        S1h = tiny.tile([BL, 1], F32, tag="S1h")
        nc.vector.tensor_reduce(S1h, hA, mybir.AxisListType.X, OP.add)
        negmu = tiny.tile([BL, 1], F32, tag="negmuf")
        nc.vector.tensor_scalar(negmu, S1h, -1.0 / HIDDEN, None, OP.mult)
        sqf = work.tile([BL, HIDDEN], F32, tag="sq")
        Qf = tiny.tile([BL, 1], F32, tag="Qf")
        nc.scalar.activation(sqf, hA, AF.Square, accum_out=Qf)
        m2e = tiny.tile([BL, 1], F32, tag="m2ef")
        nc.vector.scalar_tensor_tensor(m2e, negmu, negmu, eps_tile, OP.mult,
                                       OP.subtract)
        ve = tiny.tile([BL, 1], F32, tag="vef")
        nc.vector.scalar_tensor_tensor(ve, Qf, 1.0 / HIDDEN, m2e, OP.mult,
                                       OP.subtract)
        rstd = _emit_rstd(nc, tiny, ve, "f", 2)
        bias2 = tiny.tile([BL, 1], F32, tag="bias2f")
        nc.vector.tensor_mul(bias2, negmu, rstd)
        ln = work.tile([BL, HIDDEN], F32, tag="ln")
        nc.scalar.activation(ln, hA, AF.Identity, bias=bias2, scale=rstd)
        if need_norm_aff:
            nc.vector.tensor_mul(ln, ln, rep["ng"])
            nc.vector.tensor_add(ln, ln, rep["nb"])
        lnT_ps = psum_f.tile([128, 4 * BL], F32, tag="fT")
        lnT = state.tile([128, 4 * BL], F32R, tag="hAT")
        for c in range(4):
            nc.tensor.transpose(
                lnT_ps[:, c * BL : (c + 1) * BL], ln[:, c * 128 : (c + 1) * 128],
                ident)
        nc.vector.tensor_copy(lnT[:], lnT_ps)
        po = psum_s.tile([BL, NAPP], F32, tag="zs")
        nc.tensor.matmul(po, ones_row[:], hb[:], start=True, stop=False)
        for c in range(4):
            nc.tensor.matmul(po, lnT[:, c * BL : (c + 1) * BL],
                             hw[:, c * NAPP : (c + 1) * NAPP],
                             start=False, stop=(c == 3))
        res = work.tile([BL, NAPP], F32, tag="res")
        nc.vector.tensor_copy(res, po)
        nc.sync.dma_start(out_d[:], res[:])

    nc.compile()
    _BUILD_CACHE[key] = nc
    return nc


def _softplus(v):
    return np.log1p(np.exp(-np.abs(v))) + np.maximum(v, 0.0)


def kernel(**inputs):
    inputs = {k: np.ascontiguousarray(np.asarray(v)) for k, v in inputs.items()}
    x = inputs["x"].astype(np.float32)
    ctxv = inputs["ctx"].astype(np.float32)
    rec_w = inputs["rec_w"].astype(np.float32)
    in_w = inputs["in_w"].astype(np.float32)
    in_b = inputs["in_b"].astype(np.float32)
    tau = inputs["tau"].astype(np.float32)
    intra_g, intra_b = inputs["intra_g"].astype(np.float32), inputs["intra_b"].astype(np.float32)
    norm_g, norm_b = inputs["norm_g"].astype(np.float32), inputs["norm_b"].astype(np.float32)
    head_w, head_b = inputs["head_w"].astype(np.float32), inputs["head_b"].astype(np.float32)
    ce_w1, ce_b1 = inputs["ce_w1"].astype(np.float32), inputs["ce_b1"].astype(np.float32)
    ce_w2, ce_b2 = inputs["ce_w2"].astype(np.float32), inputs["ce_b2"].astype(np.float32)

    B, S_in, _ = x.shape
    assert B == B_FULL, B

    tau_sp = _softplus(tau).astype(np.float32)
    a_vec = (np.float32(1.0) - np.float32(DT) / tau_sp).astype(np.float32)
    need_tau_vec = not bool(np.all(a_vec == a_vec[0]))
    need_clip = not bool(np.all(tau_sp <= 10.0) and np.all(tau_sp >= DT))
    need_intra_aff = not (np.all(intra_g == 1.0) and np.all(intra_b == 0.0))
    need_norm_aff = not (np.all(norm_g == 1.0) and np.all(norm_b == 0.0))
    gen_flags = (need_intra_aff, need_tau_vec, need_clip, need_norm_aff)
    a_val = float(a_vec[0])

    nc = _build(S_in, gen_flags, a_val)

    # ---- host-side constant prep ----
    Wd = (rec_w * np.float32(DT)).astype(np.float32)  # z = h~ @ (DT*W) + x@in_w + in_b
    wrec = np.zeros((128, 4 * 514), np.float32)
    for c in range(4):
        blk = Wd[c * 128 : (c + 1) * 128, :]
        wrec[:, c * 514 : c * 514 + 512] = blk
        wrec[:, c * 514 + 512] = blk.sum(axis=1)
    wx = np.zeros((INPUT + 1, 514), np.float32)
    wx[0:INPUT, 0:HIDDEN] = in_w
    wx[INPUT, 0:HIDDEN] = in_b
    wx[0:INPUT, HIDDEN] = in_w.sum(axis=1)
    wx[INPUT, HIDDEN] = in_b.sum()
    cw1 = np.concatenate([ce_w1, ce_b1[None, :]], axis=0).astype(np.float32)  # [7,32]
    cw2 = np.concatenate([ce_w2, ce_b2[None, :]], axis=0).astype(np.float32)  # [33,512]
    hw = np.zeros((128, 4 * NAPP), np.float32)
    for c in range(4):
        hw[:, c * NAPP : (c + 1) * NAPP] = head_w[c * 128 : (c + 1) * 128, :]
    hb = head_b[None, :].astype(np.float32)
    ident = np.eye(32, dtype=np.float32)
    ident128 = np.eye(128, dtype=np.float32)
    # 4 diag blocks (diag(a_vec) per hidden chunk): fT init = (a*I)^T @ hAT
    a_ident = np.zeros((128, 4 * 128), np.float32)
    for c in range(4):
        a_ident[:, c * 128 : (c + 1) * 128] = np.diag(a_vec[c * 128 : (c + 1) * 128])

    xt = np.transpose(x, (2, 1, 0))  # [2, S, B]
    in_maps = []
    for core in range(N_CORES):
        sl = slice(core * BL, (core + 1) * BL)
        xa = np.ones((INPUT + 1, S_in * BL), np.float32)
        xa[0:INPUT] = xt[:, :, sl].reshape(INPUT, S_in * BL)
        ctxa = np.ones((CTX + 1, BL), np.float32)
        ctxa[0:CTX] = ctxv[sl].T
        m = {
            "xa": xa, "wrec": wrec, "wx": wx, "a_ident": a_ident, "ctxa": ctxa,
            "cw1": cw1, "cw2": cw2, "hw": hw, "hb": hb, "ident": ident,
            "ident128": ident128,
        }
        if need_intra_aff:
            m["ig_rep"] = np.broadcast_to(intra_g, (BL, HIDDEN)).copy()
            m["ib_rep"] = np.broadcast_to(intra_b, (BL, HIDDEN)).copy()
        if need_norm_aff:
            m["ng_rep"] = np.broadcast_to(norm_g, (BL, HIDDEN)).copy()
            m["nb_rep"] = np.broadcast_to(norm_b, (BL, HIDDEN)).copy()
        in_maps.append(m)

    br = run_bass_kernel_spmd(nc, in_maps, core_ids=list(range(N_CORES)))
    out = np.concatenate([np.asarray(r["out"]) for r in br.results], axis=0)
    global _LAST_RUN
    _LAST_RUN = (nc, in_maps)
    return out.astype(np.float32)


_LAST_RUN = None


def profile_exec_time_ns():
    """Re-run the last kernel invocation with NTFF tracing; return exec ns."""
    if _LAST_RUN is None:
        return None
    nc, in_maps = _LAST_RUN
    br = run_bass_kernel_spmd(nc, in_maps, core_ids=list(range(N_CORES)), trace=True)
    return br.exec_time_ns
